# revision 2
# baseline (speedup 1.0000x reference)
"""Distributed Trainium2 kernel for AlternateWeaveGather (segment_reduce).

Reference computation:
    h = x @ W.T + b                      # [N, 512] linear
    out = segment_mean(h, batch, 256)    # [256, 512]

The linear layer commutes with the segment sum:
    out[s] = (segsum_x[s] @ W.T) / max(c[s], 1) + b * (c[s] > 0)

batch is sorted, so the host shards x at SEGMENT boundaries: rank j gets
exactly the rows of segments [32j, 32j+32), padded with zero rows to a
fixed P. Every rank then computes its 32 output rows entirely locally —
no collective, no cross-rank coupling. Segment counts are exact host-side
bincounts, shipped as 1/max(c,1) and b*(c>0).

On-core: stream the row shard (f32, 4MB supertiles, 32KB/partition DMA
descriptors), one-hot matmul per 16-row plane accumulates segment sums
into PSUM (the f32->bf16 high-halfword bitcast makes the bf16 operand
free), then a tiny transpose + 512x512 linear epilogue.
"""

import numpy as np

import concourse.bacc as bacc
import concourse.bass as bass
import concourse.mybir as mybir
import concourse.tile as tile
from concourse.bass_utils import run_bass_kernel_spmd

N_CORES = 8
N_ROWS = 131072
D = 512
N_SEG = 256
SEG_PER_CORE = N_SEG // N_CORES
P_MAIN = 16384          # 8 supertiles x 2048 rows
P_TAIL = 512            # one 512-row tail supertile
P = P_MAIN + P_TAIL     # padded rows per core
W_WIN = 64              # one-hot window (rel ids 0..31, trash=32)
TRASH = 32

F32 = mybir.dt.float32
I32 = mybir.dt.int32
BF16 = mybir.dt.bfloat16

N_SUP = P_MAIN // 2048  # 8 big supertiles (k=16)
# plane columns: t in 0..7 -> cols 16t..16t+15 ; tail -> cols 128..131
N_PLANES = 16 * N_SUP + 4


def build_nc():
    nc = bacc.Bacc("TRN2", target_bir_lowering=False, debug=False,
                   num_devices=N_CORES)
    x = nc.dram_tensor("x", [P_MAIN, D], F32, kind="ExternalInput")
    xt_d = nc.dram_tensor("xt_d", [P_TAIL, D], F32, kind="ExternalInput")
    batchp = nc.dram_tensor("batchp", [128, N_PLANES], F32,
                            kind="ExternalInput")
    wt = nc.dram_tensor("wt", [D, D], BF16, kind="ExternalInput")
    inv_d = nc.dram_tensor("inv_d", [SEG_PER_CORE, 1], F32,
                           kind="ExternalInput")
    bind_d = nc.dram_tensor("bind_d", [SEG_PER_CORE, D], F32,
                            kind="ExternalInput")
    out = nc.dram_tensor("out", [SEG_PER_CORE, D], F32, kind="ExternalOutput")

    iota_c = nc.inline_tensor(
        np.tile(np.arange(W_WIN, dtype=np.float32), (128, 1)).astype(
            mybir.dt.np(BF16)), name="iota_c")
    sel_c = nc.inline_tensor(
        np.eye(W_WIN, SEG_PER_CORE, dtype=np.float32).astype(
            mybir.dt.np(BF16)), name="sel_c")

    # [t, p, k, d]; per (t, p) the (16, 512) block is 32KB contiguous
    x_r = x.ap().rearrange("(t p k) d -> t p k d", p=128, k=16)
    xt_r = xt_d.ap().rearrange("(p k) d -> p k d", k=4)

    with tile.TileContext(nc) as tc:
        with tc.tile_pool(name="const", bufs=1) as const:
            iota_sb = const.tile([128, W_WIN], BF16, name="iota_sb")
            batch_sb = const.tile([128, N_PLANES], F32, name="batch_sb")
            wt_sb = const.tile([128, 4 * D], BF16, name="wt_sb")
            sel_sb = const.tile([W_WIN, SEG_PER_CORE], BF16, name="sel_sb")
            inv_sb = const.tile([SEG_PER_CORE, 1], F32, name="inv_sb")
            bind_sb = const.tile([SEG_PER_CORE, D], F32, name="bind_sb")
            # consts head the scalar queue; sync starts streaming at once
            nc.scalar.dma_start(out=iota_sb[:, :], in_=iota_c[:, :])
            nc.scalar.dma_start(out=batch_sb[:, :], in_=batchp[:, :])
            nc.scalar.dma_start(out=sel_sb[:, :], in_=sel_c[:, :])
            nc.scalar.dma_start(out=inv_sb[:, :], in_=inv_d[:, :])
            nc.scalar.dma_start(out=bind_sb[:, :], in_=bind_d[:, :])
            for i in range(4):
                nc.scalar.dma_start(out=wt_sb[:, i * D:(i + 1) * D],
                                    in_=wt[i * 128:(i + 1) * 128, :])

            with tc.tile_pool(name="xin", bufs=4) as xp, \
                 tc.tile_pool(name="ohp", bufs=12) as ohp, \
                 tc.tile_pool(name="psum_acc", bufs=1, space="PSUM") as pacc:
                ps = pacc.tile([W_WIN, D], F32, name="ps")
                qs = [nc.sync, nc.scalar]
                nq = 0

                def is_eq_mm(xtile, kk, col, start, stop):
                    oh = ohp.tile([128, W_WIN], BF16, name="oh")
                    nc.vector.tensor_scalar(
                        oh[:, :], iota_sb[:, :],
                        batch_sb[:, col:col + 1],
                        None, mybir.AluOpType.is_equal)
                    nc.tensor.matmul(ps[:, :], oh[:, :],
                                     xtile[:, kk, 1::2],
                                     start=start, stop=stop,
                                     skip_group_check=True)

                for t in range(N_SUP):
                    xt = xp.tile([128, 16, D], F32, name="xt")
                    xt_bf = xt[:, :, :].bitcast(BF16)
                    if t == N_SUP - 1:
                        # split the final big supertile so the pipeline
                        # drains per-4-plane, not per-16-plane
                        for c in range(4):
                            qs[nq].dma_start(out=xt[:, 4 * c:4 * c + 4, :],
                                             in_=x_r[t][:, 4 * c:4 * c + 4, :])
                            nq ^= 1
                    else:
                        qs[nq].dma_start(out=xt[:, :, :], in_=x_r[t])
                        nq ^= 1
                    for kk in range(16):
                        is_eq_mm(xt_bf, kk, 16 * t + kk, t == 0 and kk == 0,
                                 False)

                # 512-row tail supertile (padded rows have rel id TRASH)
                xtl = xp.tile([128, 4, D], F32, name="xtl")
                xtl_bf = xtl[:, :, :].bitcast(BF16)
                qs[nq].dma_start(out=xtl[:, :, :], in_=xt_r[:, :, :])
                nq ^= 1
                for kk in range(4):
                    is_eq_mm(xtl_bf, kk, 16 * N_SUP + kk, False, kk == 3)

                with tc.tile_pool(name="epi", bufs=1) as epi, \
                     tc.tile_pool(name="psum_epi", bufs=1,
                                  space="PSUM") as pepi:
                    # segment sums live in ps rows 0..31 (32=trash,
                    # 33..63 exact zeros); truncate to bf16 in SBUF
                    sb_bf = epi.tile([W_WIN, D], BF16, name="sb_bf")
                    nc.vector.tensor_copy(sb_bf[:, :], ps[:, :])

                    # transpose via sel matmul: pt_c[d_c, s] =
                    #   sum_p sb_bf[p, d_c] * (p == s)
                    lhsT = epi.tile([128, 4 * SEG_PER_CORE], BF16,
                                    name="lhsT")
                    for c in range(4):
                        pt = pepi.tile([128, SEG_PER_CORE], F32, name="pt",
                                       tag="pt", bufs=2)
                        nc.tensor.matmul(pt[:, :],
                                         sb_bf[:, c * 128:(c + 1) * 128],
                                         sel_sb[:, :], start=True, stop=True)
                        eng = nc.vector if c % 2 == 0 else nc.scalar
                        eng_copy = (nc.vector.tensor_copy if c % 2 == 0
                                    else nc.scalar.copy)
                        eng_copy(
                            lhsT[:, c * SEG_PER_CORE:(c + 1) * SEG_PER_CORE],
                            pt[:, :])

                    po = pepi.tile([SEG_PER_CORE, D], F32, name="po")
                    for c in range(4):
                        nc.tensor.matmul(
                            po[:, :],
                            lhsT[:, c * SEG_PER_CORE:(c + 1) * SEG_PER_CORE],
                            wt_sb[:, c * D:(c + 1) * D],
                            start=(c == 0), stop=(c == 3))
                    res = epi.tile([SEG_PER_CORE, D], F32, name="res")
                    # res = (sums @ Wt) * inv + b*(c>0)
                    nc.vector.scalar_tensor_tensor(
                        res[:, :], po[:, :], inv_sb[:, 0:1],
                        bind_sb[:, :], mybir.AluOpType.mult,
                        mybir.AluOpType.add)
                    nc.sync.dma_start(out=out[:, :], in_=res[:, :])
    nc.compile()
    return nc


def make_in_maps(x, W, b, batch):
    x = np.asarray(x, dtype=np.float32)
    W = np.asarray(W, dtype=np.float32)
    b = np.asarray(b, dtype=np.float32)
    batch = np.asarray(batch).astype(np.int64)
    wt = np.ascontiguousarray(W.T).astype(mybir.dt.np(BF16))
    counts = np.bincount(batch, minlength=N_SEG).astype(np.float32)
    bounds = np.searchsorted(batch, np.arange(0, N_SEG + 1, SEG_PER_CORE))

    in_maps = []
    for j in range(N_CORES):
        lo, hi = int(bounds[j]), int(bounds[j + 1])
        n = hi - lo
        assert n <= P, f"core {j}: {n} rows exceed padded capacity {P}"
        xj = np.zeros((P, D), dtype=np.float32)
        xj[:n] = x[lo:hi]
        rel = np.full((P,), TRASH, dtype=np.float32)
        rel[:n] = (batch[lo:hi] - j * SEG_PER_CORE).astype(np.float32)
        # plane layout: main t<8, k=16: row = 2048t + 16p + kk
        relm = rel[:P_MAIN].reshape(N_SUP, 128, 16)
        cols = [relm[t, :, kk] for t in range(N_SUP) for kk in range(16)]
        # tail: row = 16384 + 4p + kk
        relt = rel[P_MAIN:].reshape(128, 4)
        cols += [relt[:, kk] for kk in range(4)]
        bp = np.stack(cols, axis=1)

        cj = counts[j * SEG_PER_CORE:(j + 1) * SEG_PER_CORE]
        inv = (1.0 / np.maximum(cj, 1.0)).reshape(SEG_PER_CORE, 1)
        bind = (cj > 0).astype(np.float32)[:, None] * b[None, :]
        in_maps.append({
            "x": np.ascontiguousarray(xj[:P_MAIN]),
            "xt_d": np.ascontiguousarray(xj[P_MAIN:]),
            "batchp": np.ascontiguousarray(bp.astype(np.float32)),
            "wt": wt,
            "inv_d": np.ascontiguousarray(inv.astype(np.float32)),
            "bind_d": np.ascontiguousarray(bind.astype(np.float32)),
        })
    return in_maps


_NC_CACHE = {}


def kernel(x, W, b, batch, num_segments, trace=False, trace_cores=None):
    assert int(num_segments) == N_SEG
    if "nc" not in _NC_CACHE:
        _NC_CACHE["nc"] = build_nc()
    nc = _NC_CACHE["nc"]
    in_maps = make_in_maps(x, W, b, batch)
    kw = {}
    if trace_cores is not None:
        kw["trace_cores"] = trace_cores
    res = run_bass_kernel_spmd(nc, in_maps, core_ids=list(range(N_CORES)),
                               trace=trace, **kw)
    full = np.concatenate([res.results[j]["out"] for j in range(N_CORES)],
                          axis=0)
    if trace:
        return full, res
    return full


# revision 3
# speedup vs baseline: 1.5144x; 1.5144x over previous
"""Distributed Trainium2 kernel for AlternateWeaveGather (segment_reduce).

Reference computation:
    h = x @ W.T + b                      # [N, 512] linear
    out = segment_mean(h, batch, 256)    # [256, 512]

The linear layer commutes with the segment sum:
    out[s] = (segsum_x[s] @ W.T) / max(c[s], 1) + b * (c[s] > 0)

batch is sorted, so the host shards x at SEGMENT boundaries: rank j gets
exactly the rows of segments [32j, 32j+32), padded with zero rows to a
fixed P. Every rank then computes its 32 output rows entirely locally —
no collective, no cross-rank coupling. Segment counts are exact host-side
bincounts, shipped as 1/max(c,1) and b*(c>0).

The host ships x as bf16 (the device PE consumed x as truncated bf16
anyway; host-side round-to-nearest is strictly more accurate), halving
the HBM stream. On-core: stream the row shard (2MB supertiles,
16KB/partition DMA descriptors), one-hot matmul per 16-row plane
accumulates segment sums into PSUM, then a tiny transpose + 512x512
linear epilogue.
"""

import numpy as np

import concourse.bacc as bacc
import concourse.bass as bass
import concourse.mybir as mybir
import concourse.tile as tile
from concourse.bass_utils import run_bass_kernel_spmd

N_CORES = 8
N_ROWS = 131072
D = 512
N_SEG = 256
SEG_PER_CORE = N_SEG // N_CORES
P_MAIN = 16384          # 8 supertiles x 2048 rows
P_TAIL = 512            # one 512-row tail supertile
P = P_MAIN + P_TAIL     # padded rows per core
W_WIN = 64              # one-hot window (rel ids 0..31, trash=32)
TRASH = 32

F32 = mybir.dt.float32
I32 = mybir.dt.int32
BF16 = mybir.dt.bfloat16

N_SUP = P_MAIN // 2048  # 8 big supertiles (k=16)
# plane columns: t in 0..7 -> cols 16t..16t+15 ; tail -> cols 128..131
N_PLANES = 16 * N_SUP + 4


def build_nc():
    nc = bacc.Bacc("TRN2", target_bir_lowering=False, debug=False,
                   num_devices=N_CORES)
    x = nc.dram_tensor("x", [P_MAIN, D], BF16, kind="ExternalInput")
    xt_d = nc.dram_tensor("xt_d", [P_TAIL, D], BF16, kind="ExternalInput")
    batchp = nc.dram_tensor("batchp", [128, N_PLANES], F32,
                            kind="ExternalInput")
    wt = nc.dram_tensor("wt", [D, D], BF16, kind="ExternalInput")
    inv_d = nc.dram_tensor("inv_d", [SEG_PER_CORE, 1], F32,
                           kind="ExternalInput")
    bind_d = nc.dram_tensor("bind_d", [SEG_PER_CORE, D], F32,
                            kind="ExternalInput")
    out = nc.dram_tensor("out", [SEG_PER_CORE, D], F32, kind="ExternalOutput")

    iota_c = nc.inline_tensor(
        np.tile(np.arange(W_WIN, dtype=np.float32), (128, 1)).astype(
            mybir.dt.np(BF16)), name="iota_c")
    sel_c = nc.inline_tensor(
        np.eye(W_WIN, SEG_PER_CORE, dtype=np.float32).astype(
            mybir.dt.np(BF16)), name="sel_c")

    # [t, p, k, d]; per (t, p) the (16, 512) block is 32KB contiguous
    x_r = x.ap().rearrange("(t p k) d -> t p k d", p=128, k=16)
    xt_r = xt_d.ap().rearrange("(p k) d -> p k d", k=4)

    with tile.TileContext(nc) as tc:
        with tc.tile_pool(name="const", bufs=1) as const:
            iota_sb = const.tile([128, W_WIN], BF16, name="iota_sb")
            batch_sb = const.tile([128, N_PLANES], F32, name="batch_sb")
            wt_sb = const.tile([128, 4 * D], BF16, name="wt_sb")
            sel_sb = const.tile([W_WIN, SEG_PER_CORE], BF16, name="sel_sb")
            inv_sb = const.tile([SEG_PER_CORE, 1], F32, name="inv_sb")
            bind_sb = const.tile([SEG_PER_CORE, D], F32, name="bind_sb")
            # consts head the scalar queue; sync starts streaming at once
            nc.scalar.dma_start(out=iota_sb[:, :], in_=iota_c[:, :])
            nc.scalar.dma_start(out=batch_sb[:, :], in_=batchp[:, :])
            nc.scalar.dma_start(out=sel_sb[:, :], in_=sel_c[:, :])
            nc.scalar.dma_start(out=inv_sb[:, :], in_=inv_d[:, :])
            nc.scalar.dma_start(out=bind_sb[:, :], in_=bind_d[:, :])
            for i in range(4):
                nc.scalar.dma_start(out=wt_sb[:, i * D:(i + 1) * D],
                                    in_=wt[i * 128:(i + 1) * 128, :])

            with tc.tile_pool(name="xin", bufs=5) as xp, \
                 tc.tile_pool(name="ohp", bufs=12) as ohp, \
                 tc.tile_pool(name="psum_acc", bufs=1, space="PSUM") as pacc:
                ps = pacc.tile([W_WIN, D], F32, name="ps")
                qs = [nc.sync, nc.scalar]
                nq = 0

                def is_eq_mm(xtile, kk, col, start, stop):
                    oh = ohp.tile([128, W_WIN], BF16, name="oh")
                    nc.vector.tensor_scalar(
                        oh[:, :], iota_sb[:, :],
                        batch_sb[:, col:col + 1],
                        None, mybir.AluOpType.is_equal)
                    nc.tensor.matmul(ps[:, :], oh[:, :],
                                     xtile[:, kk, :],
                                     start=start, stop=stop,
                                     skip_group_check=True)

                for t in range(N_SUP):
                    xt = xp.tile([128, 16, D], BF16, name="xt")
                    xt_bf = xt
                    if t == N_SUP - 1:
                        # split the final big supertile so the pipeline
                        # drains per-4-plane, not per-16-plane
                        for c in range(4):
                            qs[nq].dma_start(out=xt[:, 4 * c:4 * c + 4, :],
                                             in_=x_r[t][:, 4 * c:4 * c + 4, :])
                            nq ^= 1
                    else:
                        qs[nq].dma_start(out=xt[:, :, :], in_=x_r[t])
                        nq ^= 1
                    for kk in range(16):
                        is_eq_mm(xt_bf, kk, 16 * t + kk, t == 0 and kk == 0,
                                 False)

                # 512-row tail supertile (padded rows have rel id TRASH)
                xtl = xp.tile([128, 4, D], BF16, name="xtl")
                xtl_bf = xtl
                qs[nq].dma_start(out=xtl[:, :, :], in_=xt_r[:, :, :])
                nq ^= 1
                for kk in range(4):
                    is_eq_mm(xtl_bf, kk, 16 * N_SUP + kk, False, kk == 3)

                with tc.tile_pool(name="epi", bufs=1) as epi, \
                     tc.tile_pool(name="psum_epi", bufs=1,
                                  space="PSUM") as pepi:
                    # segment sums live in ps rows 0..31 (32=trash,
                    # 33..63 exact zeros); truncate to bf16 in SBUF
                    sb_bf = epi.tile([W_WIN, D], BF16, name="sb_bf")
                    nc.vector.tensor_copy(sb_bf[:, :], ps[:, :])

                    # transpose via sel matmul: pt_c[d_c, s] =
                    #   sum_p sb_bf[p, d_c] * (p == s)
                    lhsT = epi.tile([128, 4 * SEG_PER_CORE], BF16,
                                    name="lhsT")
                    for c in range(4):
                        pt = pepi.tile([128, SEG_PER_CORE], F32, name="pt",
                                       tag="pt", bufs=2)
                        nc.tensor.matmul(pt[:, :],
                                         sb_bf[:, c * 128:(c + 1) * 128],
                                         sel_sb[:, :], start=True, stop=True)
                        eng = nc.vector if c % 2 == 0 else nc.scalar
                        eng_copy = (nc.vector.tensor_copy if c % 2 == 0
                                    else nc.scalar.copy)
                        eng_copy(
                            lhsT[:, c * SEG_PER_CORE:(c + 1) * SEG_PER_CORE],
                            pt[:, :])

                    po = pepi.tile([SEG_PER_CORE, D], F32, name="po")
                    for c in range(4):
                        nc.tensor.matmul(
                            po[:, :],
                            lhsT[:, c * SEG_PER_CORE:(c + 1) * SEG_PER_CORE],
                            wt_sb[:, c * D:(c + 1) * D],
                            start=(c == 0), stop=(c == 3))
                    res = epi.tile([SEG_PER_CORE, D], F32, name="res")
                    # res = (sums @ Wt) * inv + b*(c>0)
                    nc.vector.scalar_tensor_tensor(
                        res[:, :], po[:, :], inv_sb[:, 0:1],
                        bind_sb[:, :], mybir.AluOpType.mult,
                        mybir.AluOpType.add)
                    nc.sync.dma_start(out=out[:, :], in_=res[:, :])
    nc.compile()
    return nc


def make_in_maps(x, W, b, batch):
    x = np.asarray(x, dtype=np.float32)
    W = np.asarray(W, dtype=np.float32)
    b = np.asarray(b, dtype=np.float32)
    batch = np.asarray(batch).astype(np.int64)
    wt = np.ascontiguousarray(W.T).astype(mybir.dt.np(BF16))
    counts = np.bincount(batch, minlength=N_SEG).astype(np.float32)
    bounds = np.searchsorted(batch, np.arange(0, N_SEG + 1, SEG_PER_CORE))

    in_maps = []
    for j in range(N_CORES):
        lo, hi = int(bounds[j]), int(bounds[j + 1])
        n = hi - lo
        assert n <= P, f"core {j}: {n} rows exceed padded capacity {P}"
        xj = np.zeros((P, D), dtype=mybir.dt.np(BF16))
        xj[:n] = x[lo:hi].astype(mybir.dt.np(BF16))
        rel = np.full((P,), TRASH, dtype=np.float32)
        rel[:n] = (batch[lo:hi] - j * SEG_PER_CORE).astype(np.float32)
        # plane layout: main t<8, k=16: row = 2048t + 16p + kk
        relm = rel[:P_MAIN].reshape(N_SUP, 128, 16)
        cols = [relm[t, :, kk] for t in range(N_SUP) for kk in range(16)]
        # tail: row = 16384 + 4p + kk
        relt = rel[P_MAIN:].reshape(128, 4)
        cols += [relt[:, kk] for kk in range(4)]
        bp = np.stack(cols, axis=1)

        cj = counts[j * SEG_PER_CORE:(j + 1) * SEG_PER_CORE]
        inv = (1.0 / np.maximum(cj, 1.0)).reshape(SEG_PER_CORE, 1)
        bind = (cj > 0).astype(np.float32)[:, None] * b[None, :]
        in_maps.append({
            "x": np.ascontiguousarray(xj[:P_MAIN]),
            "xt_d": np.ascontiguousarray(xj[P_MAIN:]),
            "batchp": np.ascontiguousarray(bp.astype(np.float32)),
            "wt": wt,
            "inv_d": np.ascontiguousarray(inv.astype(np.float32)),
            "bind_d": np.ascontiguousarray(bind.astype(np.float32)),
        })
    return in_maps


_NC_CACHE = {}


def kernel(x, W, b, batch, num_segments, trace=False, trace_cores=None):
    assert int(num_segments) == N_SEG
    if "nc" not in _NC_CACHE:
        _NC_CACHE["nc"] = build_nc()
    nc = _NC_CACHE["nc"]
    in_maps = make_in_maps(x, W, b, batch)
    kw = {}
    if trace_cores is not None:
        kw["trace_cores"] = trace_cores
    res = run_bass_kernel_spmd(nc, in_maps, core_ids=list(range(N_CORES)),
                               trace=trace, **kw)
    full = np.concatenate([res.results[j]["out"] for j in range(N_CORES)],
                          axis=0)
    if trace:
        return full, res
    return full
